# revision 47
# baseline (speedup 1.0000x reference)
"""BiLSTM-CRF Viterbi decode on 8 Trainium2 NeuronCores.

Data-parallel over batch: each core handles 16 of 128 sequences.

Per-core phases:
  P0 embedding gather (indirect DMA, 128 rows per DMA)
  P1 PE-transpose x_rows [tok,E] -> x_T [E,tok] (fp16)
  P2 bulk input projection xproj = Wih @ x (+bias) staged to DRAM, fp16
     weights split hi+lo (2-pass PSUM accumulate).  Backward-dir i/f gate
     lanes get -1e30 added at padded (b,t) so the bwd LSTM state stays
     exactly zero through trailing padding (no per-round masking needed).
  P3 512 fused fwd+bwd LSTM rounds, gate-dim on partitions, [128,16]
     tiles.  Whh in fp16 hi+lo (2-pass), h carried in fp16.  tanh(g) is
     computed as 2*sigmoid(2g)-1 (g-gate weights pre-scaled by 2) so one
     sigmoid covers all four gates.
  P4 emissions em = h @ W_out.T (fp16) staged to DRAM, read back b-major
  P5 Viterbi DP: pre[t][i,j] = trans[j,i]+em[t,i] precomputed in chunks
     (GpSimd), DP step = 1 add + 1 max-reduce on DVE, scores unfrozen
     (final score extracted from hist at t=len-1 via masked max)
  P6 bulk argmax of backpointers (constant-stationary matmul + DVE)
  P7 backtrace (DVE chain, one-hot dot per step)
"""

import numpy as np

import concourse.bacc as bacc
import concourse.bass as bass
import concourse.mybir as mybir
import concourse.tile as tile
from concourse.bass import IndirectOffsetOnAxis
from concourse.bass_utils import run_bass_kernel_spmd
from concourse.masks import make_identity

F32 = mybir.dt.float32
FP16 = mybir.dt.float16
I32 = mybir.dt.int32
I8 = mybir.dt.int8
Alu = mybir.AluOpType
Act = mybir.ActivationFunctionType
AxX = mybir.AxisListType.X

B, T, V, E, H, K = 128, 512, 100000, 128, 128, 9
NC = 8
Bc = B // NC          # 16 sequences per core
TOK = Bc * T          # 8192 tokens per core, flat index bt = b*T + t (b-major)
NBLK = TOK // 128     # 64 gather/transpose blocks
G4 = 4
# weight gate order: i, f, g, o (torch).  PSUM column block q per gate:
# i->0, f->1, o->2, g->3  (so sigmoid block i,f,o is contiguous per dir and
# the g block sits at the end; g is handled by the 2*sig(2x)-1 identity).
QMAP = {0: 0, 1: 1, 2: 3, 3: 2}
CH = 32               # LSTM rounds per xproj chunk
NCH = T // CH
DPCH = 32             # viterbi DP precompute chunk (steps)
AM_CH = 56            # bulk-argmax steps per chunk
AM_N = [AM_CH] * 9 + [511 - 9 * AM_CH]
NEG = -1.0e30
NEG16 = -60000.0      # "-inf" that survives fp16 (sigmoid(-6e4+x) == 0)


def build_program():
    nc = bacc.Bacc(None, target_bir_lowering=False)

    # ---------------- dram parameters ----------------
    embed = nc.declare_dram_parameter("embed", [V, E], F32, isOutput=False)
    idx = nc.declare_dram_parameter("idx", [128, NBLK], I32, isOutput=False)
    whh_hi = nc.declare_dram_parameter("whh_hi", [128, 1024], FP16, isOutput=False)
    whh_lo = nc.declare_dram_parameter("whh_lo", [128, 1024], FP16, isOutput=False)
    wih_hi = nc.declare_dram_parameter("wih_hi", [128, 1024], FP16, isOutput=False)
    wih_lo = nc.declare_dram_parameter("wih_lo", [128, 1024], FP16, isOutput=False)
    bias2 = nc.declare_dram_parameter("bias2", [2, 1024], FP16, isOutput=False)
    mneg16 = nc.declare_dram_parameter("mneg16", [1, TOK], FP16, isOutput=False)
    woutT = nc.declare_dram_parameter("woutT", [128, 18], FP16, isOutput=False)
    bout2 = nc.declare_dram_parameter("bout2", [2, K], FP16, isOutput=False)
    start_b = nc.declare_dram_parameter("start_b", [Bc, K], F32, isOutput=False)
    end_b = nc.declare_dram_parameter("end_b", [Bc, K], F32, isOutput=False)
    trans_all = nc.declare_dram_parameter("trans_all", [Bc, 81], F32, isOutput=False)
    irev9_p = nc.declare_dram_parameter("irev9", [Bc, K], F32, isOutput=False)
    iota9_p = nc.declare_dram_parameter("iota9", [Bc, K], F32, isOutput=False)
    mask_dp = nc.declare_dram_parameter("mask_dp", [Bc, T], F32, isOutput=False)
    selneg_p = nc.declare_dram_parameter("selneg", [Bc, T], F32, isOutput=False)
    selAB = nc.declare_dram_parameter("selAB", [48, 144], F32, isOutput=False)
    trans_tiled = nc.declare_dram_parameter("trans_tiled", [K, AM_CH * K], F32, isOutput=False)
    iota_rev_am = nc.declare_dram_parameter("iota_rev_am", [72, AM_CH * K], F32, isOutput=False)
    # col 0: 1056*j (padded-step z value), col 1: 256 + 1024*j (ra2 offset)
    iota_j72 = nc.declare_dram_parameter("iota_j72", [72, 2], F32, isOutput=False)
    mask_bj = nc.declare_dram_parameter("mask_bj", [72, 2 * T], I8, isOutput=False)
    tags_out = nc.declare_dram_parameter("tags", [Bc, T], I32, isOutput=True)

    # ---------------- dram internals ----------------
    xproj_dram = nc.dram_tensor("xproj_dram", [2, G4, Bc, 128, T], FP16)

    with tile.TileContext(nc) as tc:
        with (
            tc.tile_pool(name="big", bufs=1) as big,
            tc.tile_pool(name="xp", bufs=2) as xpp,
            tc.tile_pool(name="consts", bufs=1) as cst,
            tc.tile_pool(name="small", bufs=2) as sm,
            tc.tile_pool(name="pre", bufs=2) as prep,
        ):
            # ---------- constants ----------
            idx_sb = cst.tile([128, NBLK], I32)
            nc.sync.dma_start(out=idx_sb[:], in_=idx[:])
            whhhi_sb = cst.tile([128, 1024], FP16)
            nc.sync.dma_start(out=whhhi_sb[:], in_=whh_hi[:])
            whhlo_sb = cst.tile([128, 1024], FP16)
            nc.sync.dma_start(out=whhlo_sb[:], in_=whh_lo[:])
            wihhi_sb = cst.tile([128, 1024], FP16)
            nc.sync.dma_start(out=wihhi_sb[:], in_=wih_hi[:])
            wihlo_sb = cst.tile([128, 1024], FP16)
            nc.sync.dma_start(out=wihlo_sb[:], in_=wih_lo[:])
            bias_sb = cst.tile([2, 1024], FP16)
            nc.sync.dma_start(out=bias_sb[:], in_=bias2[:])
            mneg_sb = cst.tile([1, TOK], FP16)
            nc.sync.dma_start(out=mneg_sb[:], in_=mneg16[:])
            ones2 = cst.tile([2, 512], FP16)
            nc.vector.memset(ones2[:], 1.0)
            woutT_sb = cst.tile([128, 18], FP16)
            nc.sync.dma_start(out=woutT_sb[:], in_=woutT[:])
            bout_sb = cst.tile([2, K], FP16)
            nc.sync.dma_start(out=bout_sb[:], in_=bout2[:])
            ident = cst.tile([128, 128], F32)
            make_identity(nc, ident[:])
            ident16 = cst.tile([128, 128], FP16)
            nc.vector.tensor_copy(out=ident16[:], in_=ident[:])

            # PE "absorber" ops: self-loading matmuls may carry at most ONE
            # sync wait in walrus codegen.  These tiny ops advance PE's
            # vector clock over one-time deps (identity from Pool,
            # const-weight DMA lanes) so real matmuls each need <=1 wait.
            psp_cm = tc.tile_pool(name="psglob", bufs=1, space="PSUM")
            psp = psp_cm.__enter__()
            pq1 = psp.tile([128, 128], F32, tag="pq1", name="pq1")
            pq2 = psp.tile([128, 128], F32, tag="pq2", name="pq2")
            pw1 = psp.tile([128, 512], F32, tag="pw1", name="pw1")
            pw2 = psp.tile([128, 512], F32, tag="pw2", name="pw2")
            pw3 = psp.tile([128, 512], F32, tag="pw3", name="pw3")
            nc.tensor.transpose(out=pq1[:], in_=ident[:], identity=ident[:])
            for cst_ap in (wihhi_sb[:, 0:1], wihlo_sb[:, 0:1],
                           whhhi_sb[:, 0:1], whhlo_sb[:, 0:1],
                           woutT_sb[:, 0:1], ident16[:, 0:1]):
                nc.tensor.matmul(out=pq2[0:1, 0:1], lhsT=cst_ap,
                                 rhs=cst_ap, start=True, stop=True)
            for cst_ap in (bias_sb[:, 0:1], mneg_sb[:, 0:1], ones2[:, 0:1],
                           bout_sb[:, 0:1]):
                nc.tensor.matmul(out=pq2[0:1, 0:1], lhsT=cst_ap,
                                 rhs=cst_ap, start=True, stop=True)

            # ---------- P0: gather ----------
            x_rows = []
            with tc.tile_pool(name="xr", bufs=12) as xrp:
                for g in range(NBLK):
                    xr = xrp.tile([128, 128], F32, tag="xr")
                    nc.gpsimd.indirect_dma_start(
                        out=xr[:],
                        out_offset=None,
                        in_=embed[:],
                        in_offset=IndirectOffsetOnAxis(
                            ap=idx_sb[:, g:g + 1], axis=0),
                    )
                    x_rows.append(xr)

                # ---------- P1: transpose ----------
                # All PE-facing producers routed through DVE so each
                # self-loading matmul needs a single sync wait (walrus limit):
                # relay gathered blocks DVE, output copies DVE (fp16).
                x_T = big.tile([128, TOK], FP16, tag="xT")
                with tc.tile_pool(name="xrel", bufs=4) as xrelp:
                    psts = [pq1, pq2]
                    for g in range(NBLK):
                        xrel = xrelp.tile([128, 128], F32, tag="xrel")
                        nc.vector.tensor_tensor(
                            out=xrel[:], in0=x_rows[g][:], in1=x_rows[g][:],
                            op=Alu.max)
                        pst = psts[g % 2]
                        nc.tensor.transpose(
                            out=pst[:], in_=xrel[:], identity=ident[:])
                        nc.vector.tensor_copy(
                            out=x_T[:, g * 128:(g + 1) * 128], in_=pst[:])

            # ---------- P2: bulk xproj ----------
            ps2s = [pw1[:], pw2[:], pw3[:]]
            n2 = 0
            for d in range(2):
                for b in range(Bc):
                    for g in range(G4):
                        blk = slice((d * G4 + g) * 128, (d * G4 + g + 1) * 128)
                        ps2 = ps2s[n2 % 3]
                        n2 += 1
                        nc.tensor.matmul(
                            out=ps2, lhsT=wihhi_sb[:, blk],
                            rhs=x_T[:, b * T:(b + 1) * T],
                            start=True, stop=False, skip_group_check=True)
                        nc.tensor.matmul(
                            out=ps2, lhsT=wihlo_sb[:, blk],
                            rhs=x_T[:, b * T:(b + 1) * T],
                            start=False, stop=False, skip_group_check=True)
                        # bias via contract-1 matmul row
                        last = not (d == 1 and g in (0, 1))
                        nc.tensor.matmul(
                            out=ps2, lhsT=bias_sb[:, blk], rhs=ones2[:],
                            start=False, stop=last, skip_group_check=True)
                        if not last:  # bwd i,f: freeze padding via mneg row
                            nc.tensor.matmul(
                                out=ps2, lhsT=ones2[0:1, 0:128],
                                rhs=mneg_sb[:, b * T:(b + 1) * T],
                                start=False, stop=True, skip_group_check=True)
                        xp_sb = sm.tile([128, 512], FP16, tag="xp_out")
                        nc.scalar.copy(out=xp_sb[:], in_=ps2)
                        nc.sync.dma_start(
                            out=xproj_dram[d, QMAP[g], b], in_=xp_sb[:])

            # ---------- P3: LSTM ----------
            h_f = big.tile([128, TOK], FP16, tag="hf")
            h_b = big.tile([128, TOK], FP16, tag="hb")
            h0 = cst.tile([128, Bc], FP16)
            nc.vector.memset(h0[:], 0.0)
            c_st = cst.tile([128, 2 * Bc], F32)
            nc.vector.memset(c_st[:], 0.0)

            ps3d = {0: pq1, 1: pq2}
            xp_tiles = {}
            for r in range(T):
                tf, tb = r, T - 1 - r
                c = r // CH
                if r % CH == 0:
                    for d, cc in ((0, c), (1, NCH - 1 - c)):
                        xt = xpp.tile([128, G4 * Bc * CH], FP16, tag=f"xpc{d}")
                        src = xproj_dram[d][:, :, :, cc * CH:(cc + 1) * CH]
                        src = src.transpose([2, 0, 1, 3])
                        dst = xt[:].rearrange(
                            "p (q b t) -> p q b t", q=G4, b=Bc, t=CH)
                        nc.sync.dma_start(out=dst, in_=src)
                        xp_tiles[d] = xt

                sig = sm.tile([128, 128], F32, tag="sig")
                for d, tt in ((0, tf), (1, tb)):
                    ps3 = ps3d[d]
                    if r == 0:
                        hprev = h0[:]
                    elif d == 0:
                        hprev = h_f[:, tf - 1::T]
                    else:
                        hprev = h_b[:, tb + 1::T]
                    # xproj(+bias, +pad mask) seeds PSUM via identity matmul
                    # (single start=True per bank per round; gates accumulate)
                    xsl = xp_tiles[d][:].rearrange(
                        "p (q b t) -> p q b t", q=G4, b=Bc, t=CH
                    )[:, :, :, tt % CH]
                    nc.tensor.matmul(
                        out=ps3[:, 0:64], lhsT=ident16[:],
                        rhs=xsl, start=True, stop=False, skip_group_check=True)
                    for g in range(G4):
                        blk = slice((d * G4 + g) * 128, (d * G4 + g + 1) * 128)
                        col = QMAP[g] * 16
                        nc.tensor.matmul(
                            out=ps3[:, col:col + Bc],
                            lhsT=whhhi_sb[:, blk], rhs=hprev,
                            start=False, stop=False, skip_group_check=True)
                        nc.tensor.matmul(
                            out=ps3[:, col:col + Bc],
                            lhsT=whhlo_sb[:, blk], rhs=hprev,
                            start=False, stop=(g == G4 - 1),
                            skip_group_check=True)
                    # sigmoid over this dir's gates; g block holds sigma(2x)
                    nc.scalar.activation(
                        out=sig[:, d * 64:(d + 1) * 64], in_=ps3[:, 0:64],
                        func=Act.Sigmoid)
                sigv = sig[:].rearrange("p (d q b) -> p d q b", d=2, q=4)
                # tg = 2*sig_g - 1  (= tanh of pre-scaled g)
                tg = sm.tile([128, 2 * Bc], F32, tag="tg")
                nc.vector.tensor_scalar(
                    out=tg[:].rearrange("p (d b) -> p d b", d=2),
                    in0=sigv[:, :, 3], scalar1=2.0, scalar2=-1.0,
                    op0=Alu.mult, op1=Alu.add)
                t1 = sm.tile([128, 2 * Bc], F32, tag="t1")
                nc.vector.tensor_tensor(
                    out=t1[:].rearrange("p (d b) -> p d b", d=2),
                    in0=sigv[:, :, 0],
                    in1=tg[:].rearrange("p (d b) -> p d b", d=2), op=Alu.mult)
                t2 = sm.tile([128, 2 * Bc], F32, tag="t2")
                nc.gpsimd.tensor_tensor(
                    out=t2[:].rearrange("p (d b) -> p d b", d=2),
                    in0=sigv[:, :, 1],
                    in1=c_st[:].rearrange("p (d b) -> p d b", d=2),
                    op=Alu.mult)
                nc.vector.tensor_tensor(
                    out=c_st[:], in0=t1[:], in1=t2[:], op=Alu.add)
                tcx = sm.tile([128, 2 * Bc], F32, tag="tc")
                nc.scalar.activation(out=tcx[:], in_=c_st[:], func=Act.Tanh)
                nc.vector.tensor_tensor(
                    out=h_f[:, tf::T], in0=sigv[:, 0, 2],
                    in1=tcx[:, 0:Bc], op=Alu.mult)
                nc.gpsimd.tensor_tensor(
                    out=h_b[:, tb::T], in0=sigv[:, 1, 2],
                    in1=tcx[:, Bc:2 * Bc], op=Alu.mult)

            # ---------- P4: emissions (staged straight into em_dp) ----------
            em_dp = big.tile([Bc, T * K], F32, tag="em_dp")
            ps4s = [pq2[:, 0:K], pq1[:, 0:K]]
            for ch in range(NBLK):
                ps4 = ps4s[ch % 2]
                nc.tensor.matmul(
                    out=ps4, lhsT=h_f[:, ch * 128:(ch + 1) * 128],
                    rhs=woutT_sb[:, 0:K], start=True, stop=False,
                    skip_group_check=True)
                nc.tensor.matmul(
                    out=ps4, lhsT=h_b[:, ch * 128:(ch + 1) * 128],
                    rhs=woutT_sb[:, K:2 * K], start=False, stop=False,
                    skip_group_check=True)
                nc.tensor.matmul(
                    out=ps4, lhsT=ones2[:, 0:128], rhs=bout_sb[:],
                    start=False, stop=True, skip_group_check=True)
                em_sb = sm.tile([128, K], F32, tag="em_sb")
                nc.scalar.copy(out=em_sb[:], in_=ps4)
                b_ch, t0 = ch // 4, (ch % 4) * 128
                nc.sync.dma_start(
                    out=em_dp[b_ch:b_ch + 1, t0 * K:(t0 + 128) * K],
                    in_=em_sb[:])

            # ---------- P5: Viterbi DP ----------
            trans_sb = cst.tile([Bc, 81], F32)
            nc.sync.dma_start(out=trans_sb[:], in_=trans_all[:])
            irev9_sb = cst.tile([Bc, K], F32)
            nc.sync.dma_start(out=irev9_sb[:], in_=irev9_p[:])
            iota9_sb = cst.tile([Bc, K], F32)
            nc.sync.dma_start(out=iota9_sb[:], in_=iota9_p[:])
            start_sb = cst.tile([Bc, K], F32)
            nc.sync.dma_start(out=start_sb[:], in_=start_b[:])
            end_sb = cst.tile([Bc, K], F32)
            nc.sync.dma_start(out=end_sb[:], in_=end_b[:])
            mask_sb = cst.tile([Bc, T], F32)
            nc.sync.dma_start(out=mask_sb[:], in_=mask_dp[:])
            selneg_sb = cst.tile([Bc, T], F32)
            nc.sync.dma_start(out=selneg_sb[:], in_=selneg_p[:])

            # hist slot t = S_t (unfrozen), t = 0..511
            hist = big.tile([Bc, T * K], F32, tag="hist")
            cand = cst.tile([Bc, 81], F32)
            nc.vector.tensor_tensor(
                out=hist[:, 0:K], in0=em_dp[:, 0:K], in1=start_sb[:],
                op=Alu.add)
            # pre chunks on GpSimd: pre[t][i,j] = trans[j,i] + em[t,i]
            pre_tiles = []
            for pc in range(T // DPCH):
                t0 = pc * DPCH
                n = DPCH if pc > 0 else DPCH - 1  # steps t0..t0+n-1, skip t=0
                s0 = t0 if pc > 0 else 1
                pr = prep.tile([Bc, DPCH * 81], F32, tag="pre")
                nc.vector.tensor_tensor(
                    out=pr[:, 0:n * 81].rearrange(
                        "p (t i j) -> p t i j", i=K, j=K),
                    in0=em_dp[:, s0 * K:(s0 + n) * K].rearrange(
                        "p (t i) -> p t i", i=K).unsqueeze(3).to_broadcast(
                        [Bc, n, K, K]),
                    in1=trans_sb[:].rearrange(
                        "p (i j) -> p i j", i=K).unsqueeze(1).to_broadcast(
                        [Bc, n, K, K]),
                    op=Alu.add)
                pre_tiles.append((pr, s0, n))
                for si in range(n):
                    t = s0 + si
                    nc.vector.tensor_tensor(
                        out=cand[:].rearrange("p (i j) -> p i j", i=K),
                        in0=hist[:, (t - 1) * K:t * K].unsqueeze(1)
                        .to_broadcast([Bc, K, K]),
                        in1=pr[:, si * 81:(si + 1) * 81].rearrange(
                            "p (i j) -> p i j", i=K),
                        op=Alu.add)
                    nc.vector.tensor_reduce(
                        out=hist[:, t * K:(t + 1) * K],
                        in_=cand[:].rearrange("p (i j) -> p i j", i=K),
                        axis=AxX, op=Alu.max)

            # final score: S_fin[b,i] = hist[b, len-1, i] via masked max
            tmps = big.tile([Bc, T * K], F32, tag="em_dp")
            nc.vector.tensor_tensor(
                out=tmps[:].rearrange("p (t i) -> p t i", i=K),
                in0=hist[:].rearrange("p (t i) -> p t i", i=K),
                in1=selneg_sb[:].unsqueeze(2).to_broadcast([Bc, T, K]),
                op=Alu.add)
            S = cst.tile([Bc, K], F32)
            nc.vector.tensor_reduce(
                out=S[:], in_=tmps[:].rearrange("p (t i) -> p i t", i=K),
                axis=AxX, op=Alu.max)

            tags_f = big.tile([Bc, T], F32, tag="tags_f")
            nc.vector.tensor_tensor(
                out=S[:], in0=S[:], in1=end_sb[:], op=Alu.add)
            m1 = sm.tile([Bc, 1], F32, tag="m1")
            nc.vector.tensor_reduce(out=m1[:], in_=S[:], axis=AxX, op=Alu.max)
            eqv = sm.tile([Bc, K], F32, tag="eqv")
            nc.vector.tensor_tensor(
                out=eqv[:], in0=S[:], in1=m1[:].to_broadcast([Bc, K]),
                op=Alu.is_equal)
            nc.vector.tensor_tensor(
                out=eqv[:], in0=eqv[:], in1=irev9_sb[:], op=Alu.mult)
            r1 = sm.tile([Bc, 1], F32, tag="r1")
            nc.vector.tensor_reduce(out=r1[:], in_=eqv[:], axis=AxX, op=Alu.max)
            # tags_f carries 1024*tag during the backtrace
            nc.vector.tensor_scalar(
                out=tags_f[:, T - 1:T], in0=r1[:], scalar1=-1024.0,
                scalar2=8192.0, op0=Alu.mult, op1=Alu.add)

            # ---------- P6: bulk argmax ----------
            selAB_dma = cst.tile([48, 144], F32)
            nc.sync.dma_start(out=selAB_dma[:], in_=selAB[:])
            selAB_sb = cst.tile([48, 144], F32)
            nc.vector.tensor_copy(out=selAB_sb[:], in_=selAB_dma[:])
            ttl_dma = cst.tile([K, AM_CH * K], F32)
            nc.sync.dma_start(out=ttl_dma[:], in_=trans_tiled[:])
            iram_sb = cst.tile([72, AM_CH * K], F32)
            nc.sync.dma_start(out=iram_sb[:], in_=iota_rev_am[:])
            ij72_sb = cst.tile([72, 2], F32)
            nc.sync.dma_start(out=ij72_sb[:], in_=iota_j72[:])
            mask_bj_sb = cst.tile([72, 2 * T], I8)
            nc.sync.dma_start(out=mask_bj_sb[:], in_=mask_bj[:])
            Rrhs = cst.tile([48, AM_CH * K], F32)
            nc.vector.memset(Rrhs[:], 0.0)
            nc.vector.tensor_copy(out=Rrhs[0:K, :], in_=ttl_dma[:])

            # idx_dp: [b, (j, s)] layout, s = 0..510 for steps t = 1..511.
            # Overlays the em_dp buffer (em no longer needed; tmps done).
            idx_full = big.tile([Bc, T * K], F32, tag="em_dp")

            psA = pw1[0:72, 0:AM_CH * K]
            psB = pw2[0:72, 0:AM_CH * K]
            s0 = 0
            for ci, ns in enumerate(AM_N):
                W = ns * K
                nc.vector.tensor_tensor(
                    out=Rrhs[32:48, 0:W],
                    in0=hist[:, s0 * K:(s0 + ns) * K],
                    in1=hist[:, s0 * K:(s0 + ns) * K], op=Alu.max)
                nc.tensor.matmul(out=psA[:, 0:W], lhsT=selAB_sb[:, 0:72],
                                 rhs=Rrhs[:, 0:W], start=True, stop=True)
                nc.tensor.matmul(out=psB[:, 0:W], lhsT=selAB_sb[:, 72:144],
                                 rhs=Rrhs[:, 0:W], start=True, stop=True)

                for hi, psx in ((0, psA), (1, psB)):
                    half = "AB"[hi]
                    sbx = sm.tile([72, AM_CH * K], F32, tag=f"sb{half}")
                    nc.vector.tensor_copy(out=sbx[:, 0:W], in_=psx[:, 0:W])
                    view = sbx[:, 0:W].rearrange("p (t i) -> p t i", i=K)
                    mxa = sm.tile([72, AM_CH], F32, tag=f"mx{half}")
                    nc.vector.tensor_reduce(
                        out=mxa[:, 0:ns], in_=view, axis=AxX, op=Alu.max)
                    eqa = sm.tile([72, AM_CH * K], F32, tag=f"eq{half}")
                    nc.vector.tensor_tensor(
                        out=eqa[:, 0:W].rearrange("p (t i) -> p t i", i=K),
                        in0=view,
                        in1=mxa[:, 0:ns].unsqueeze(2).to_broadcast(
                            [72, ns, K]),
                        op=Alu.is_equal)
                    nc.gpsimd.tensor_tensor(
                        out=eqa[:, 0:W], in0=eqa[:, 0:W],
                        in1=iram_sb[:, 0:W], op=Alu.mult)
                    ra = sm.tile([72, AM_CH], F32, tag=f"r{half}")
                    nc.vector.tensor_reduce(
                        out=ra[:, 0:ns],
                        in_=eqa[:, 0:W].rearrange("p (t i) -> p t i", i=K),
                        axis=AxX, op=Alu.max)
                    # z = 32*idx + 1024*j; idx = 8-ra where valid, j at pads
                    ia = sm.tile([72, AM_CH], F32, tag=f"i{half}")
                    nc.vector.tensor_tensor(
                        out=ia[:, 0:ns],
                        in0=ij72_sb[:, 0:1].to_broadcast([72, ns]),
                        in1=ij72_sb[:, 0:1].to_broadcast([72, ns]), op=Alu.max)
                    ra2 = sm.tile([72, AM_CH], F32, tag=f"r2{half}")
                    nc.vector.tensor_scalar(
                        out=ra2[:, 0:ns], in0=ra[:, 0:ns], scalar1=-32.0,
                        scalar2=ij72_sb[:, 1:2], op0=Alu.mult, op1=Alu.add)
                    nc.vector.copy_predicated(
                        out=ia[:, 0:ns],
                        mask=mask_bj_sb[:, hi * T + s0 + 1:
                                        hi * T + s0 + 1 + ns],
                        data=ra2[:, 0:ns])
                    # regroup [(b,j), t] -> [b, (j, s0+t)] via sbuf dma
                    nc.sync.dma_start(
                        out=idx_full[hi * 8:(hi + 1) * 8, 0:K * 511].rearrange(
                            "p (j s) -> p j s", j=K)[:, :, s0:s0 + ns],
                        in_=ia[:, 0:ns])
                s0 += ns

            # ---------- P7: backtrace ----------
            # z[b,j,s] = 32*idx + 1024*j.  u = 32*(z - 1024*tag_{s+1}) has
            # |u| = 1024*idx at j == tag and |u| >= 24576 elsewhere, so an
            # abs-min reduce yields 1024*tag_s directly: 2 ops per step.
            oh = sm.tile([Bc, K], F32, tag="oh")
            for s in range(T - 2, -1, -1):
                nc.vector.tensor_scalar(
                    out=oh[:],
                    in0=idx_full[:, 0:K * 511].rearrange(
                        "p (j s) -> p j s", j=K)[:, :, s],
                    scalar1=tags_f[:, s + 1:s + 2], scalar2=32.0,
                    op0=Alu.subtract, op1=Alu.mult)
                nc.vector.tensor_reduce(
                    out=tags_f[:, s:s + 1], in_=oh[:], axis=AxX, op=Alu.min,
                    apply_absolute_value=True)
            nc.vector.tensor_tensor(
                out=tags_f[:], in0=tags_f[:], in1=mask_sb[:], op=Alu.mult)
            tags_i = big.tile([Bc, T], I32, tag="tags_i")
            nc.vector.tensor_scalar(
                out=tags_i[:], in0=tags_f[:], scalar1=1.0 / 1024.0,
                scalar2=None, op0=Alu.mult)
            nc.sync.dma_start(out=tags_out[:], in_=tags_i[:])
            psp_cm.__exit__(None, None, None)

    nc.finalize()
    return nc


_NC_CACHE = None


def _get_program():
    global _NC_CACHE
    if _NC_CACHE is None:
        _NC_CACHE = build_program()
    return _NC_CACHE


def _fp16(x):
    return np.asarray(x, np.float16)


def make_in_maps(sentences, lengths, embed, Wih_f, Whh_f, bih_f, bhh_f,
                 Wih_b, Whh_b, bih_b, bhh_b, W_out, b_out, start_t, end_t,
                 trans):
    sentences = np.ascontiguousarray(sentences, dtype=np.int32)
    embed = np.ascontiguousarray(embed, dtype=np.float32)
    lengths = np.asarray(lengths)

    # g-gate rows (2H..3H in torch i,f,g,o packing) pre-scaled by 2 for the
    # tanh(z) = 2*sigmoid(2z) - 1 single-activation trick.
    def gscale(w):
        w = np.asarray(w, np.float32).copy()
        w[2 * H:3 * H] *= 2.0
        return w

    whh_hi = np.zeros((128, 1024), np.float16)
    whh_lo = np.zeros((128, 1024), np.float16)
    wih_hi = np.zeros((128, 1024), np.float16)
    wih_lo = np.zeros((128, 1024), np.float16)
    bias2_np = np.zeros((2, 1024), np.float16)
    for d, (Wih, Whh, bi, bh) in enumerate(
            ((Wih_f, Whh_f, bih_f, bhh_f), (Wih_b, Whh_b, bih_b, bhh_b))):
        Wihs, Whhs = gscale(Wih), gscale(Whh)
        bsum = gscale((np.asarray(bi) + np.asarray(bh))[:, None])[:, 0]
        for g in range(G4):
            cols = slice((d * G4 + g) * 128, (d * G4 + g + 1) * 128)
            wh = Whhs[g * 128:(g + 1) * 128, :].T
            hi = _fp16(wh)
            whh_hi[:, cols] = hi
            whh_lo[:, cols] = _fp16(wh - np.asarray(hi, np.float32))
            wi = Wihs[g * 128:(g + 1) * 128, :].T
            hi = _fp16(wi)
            wih_hi[:, cols] = hi
            wih_lo[:, cols] = _fp16(wi - np.asarray(hi, np.float32))
            bseg = bsum[g * 128:(g + 1) * 128]
            bhi = _fp16(bseg)
            bias2_np[0, cols] = bhi
            bias2_np[1, cols] = _fp16(bseg - np.asarray(bhi, np.float32))

    W_out = np.asarray(W_out, np.float32)
    woutT = np.zeros((128, 18), np.float16)
    woutT[:, 0:K] = _fp16(W_out[:, :128].T)
    woutT[:, K:2 * K] = _fp16(W_out[:, 128:].T)
    bvec = np.asarray(b_out, np.float32)
    bout_hi = _fp16(bvec)
    bout2_np = np.stack(
        [bout_hi, _fp16(bvec - np.asarray(bout_hi, np.float32))], axis=0)
    start_bc = np.broadcast_to(
        np.asarray(start_t, np.float32)[None, :], (Bc, K)).copy()
    end_bc = np.broadcast_to(
        np.asarray(end_t, np.float32)[None, :], (Bc, K)).copy()

    trans_np = np.asarray(trans, np.float32)
    trans_flat = trans_np.T.reshape(-1)  # [(i,j)] = trans[j,i]
    trans_allv = np.broadcast_to(trans_flat[None, :], (Bc, 81)).copy()
    ii = np.arange(K, dtype=np.float32)
    irev9 = np.broadcast_to((8.0 - ii)[None, :], (Bc, K)).copy()
    iota9_np = np.broadcast_to(ii[None, :], (Bc, K)).copy()

    selAB_np = np.zeros((48, 144), np.float32)
    for half in range(2):
        for m in range(72):
            b_loc, j = divmod(m, K)
            selAB_np[32 + half * 8 + b_loc, half * 72 + m] = 1.0
            selAB_np[j, half * 72 + m] = 1.0
    ttl = np.zeros((K, AM_CH * K), np.float32)
    for jp in range(K):
        ttl[jp] = np.tile(trans_np[:, jp], AM_CH)
    iram = np.broadcast_to(
        np.tile(8.0 - ii, AM_CH)[None, :], (72, AM_CH * K)).copy()
    j72 = np.arange(72, dtype=np.float32) % K
    ij72 = np.stack([1056.0 * j72, 256.0 + 1024.0 * j72], axis=1).copy()
    tt = np.arange(T)

    in_maps = []
    for c in range(NC):
        sl = slice(c * Bc, (c + 1) * Bc)
        sents_c = sentences[sl]
        lens_c = np.asarray(lengths[sl], np.float32)
        idx_np = np.zeros((128, NBLK), np.int32)
        p = np.arange(128)
        for g in range(NBLK):
            bt = g * 128 + p
            idx_np[:, g] = sents_c[bt // T, bt % T]
        mask_np = (tt[None, :] < lens_c[:, None]).astype(np.float32)
        # mneg[(b,t)] = 0 if t < len[b] else -6e4 (fp16)
        mneg16_np = np.where(
            mask_np > 0, 0.0, NEG16).astype(np.float16).reshape(1, TOK)
        selneg_np = np.where(
            tt[None, :] == (lens_c[:, None] - 1), 0.0, NEG).astype(np.float32)
        # mask_bj[(half, b_loc, j), t] = mask[half*8 + b_loc, t]
        mbj = np.repeat(mask_np, K, axis=0).astype(np.int8)  # [144, T]
        mask_bj_np = np.concatenate([mbj[:72], mbj[72:]], axis=1)  # [72, 2T]
        in_maps.append({
            "embed": embed,
            "idx": idx_np,
            "whh_hi": whh_hi, "whh_lo": whh_lo,
            "wih_hi": wih_hi, "wih_lo": wih_lo,
            "bias2": bias2_np,
            "mneg16": mneg16_np,
            "woutT": woutT, "bout2": bout2_np,
            "start_b": start_bc, "end_b": end_bc,
            "trans_all": trans_allv, "irev9": irev9, "iota9": iota9_np,
            "mask_dp": mask_np, "selneg": selneg_np,
            "selAB": selAB_np, "trans_tiled": ttl, "iota_rev_am": iram,
            "iota_j72": ij72, "mask_bj": mask_bj_np,
        })
    return in_maps


def run(inputs, trace=False, **kw):
    nc = _get_program()
    in_maps = make_in_maps(**inputs)
    res = run_bass_kernel_spmd(nc, in_maps, list(range(NC)), trace=trace, **kw)
    tags = np.concatenate([r["tags"] for r in res.results], axis=0)
    return tags.astype(np.int32), res


def kernel(**inputs):
    tags, _ = run(inputs)
    return tags


# revision 49
# speedup vs baseline: 1.1300x; 1.1300x over previous
"""BiLSTM-CRF Viterbi decode on 8 Trainium2 NeuronCores.

Data-parallel over batch: each core handles 16 of 128 sequences.

Per-core phases:
  P0 embedding gather (indirect DMA, 128 rows per DMA)
  P1 PE-transpose x_rows [tok,E] -> x_T [E,tok] (fp16)
  P2 bulk input projection xproj = Wih @ x (+bias) staged to DRAM, fp16
     weights split hi+lo (2-pass PSUM accumulate).  Backward-dir i/f gate
     lanes get -1e30 added at padded (b,t) so the bwd LSTM state stays
     exactly zero through trailing padding (no per-round masking needed).
  P3 512 fused fwd+bwd LSTM rounds, gate-dim on partitions, [128,16]
     tiles.  Whh in fp16 hi+lo (2-pass), h carried in fp16.  tanh(g) is
     computed as 2*sigmoid(2g)-1 (g-gate weights pre-scaled by 2) so one
     sigmoid covers all four gates.
  P4 emissions em = h @ W_out.T (fp16) staged to DRAM, read back b-major
  P5 Viterbi DP: pre[t][i,j] = trans[j,i]+em[t,i] precomputed in chunks
     (GpSimd), DP step = 1 add + 1 max-reduce on DVE, scores unfrozen
     (final score extracted from hist at t=len-1 via masked max)
  P6 bulk argmax of backpointers (constant-stationary matmul + DVE)
  P7 backtrace (DVE chain, one-hot dot per step)
"""

import numpy as np

import concourse.bacc as bacc
import concourse.bass as bass
import concourse.mybir as mybir
import concourse.tile as tile
from concourse.bass import IndirectOffsetOnAxis
from concourse.bass_utils import run_bass_kernel_spmd
from concourse.masks import make_identity

F32 = mybir.dt.float32
FP16 = mybir.dt.float16
I32 = mybir.dt.int32
I8 = mybir.dt.int8
Alu = mybir.AluOpType
Act = mybir.ActivationFunctionType
AxX = mybir.AxisListType.X

B, T, V, E, H, K = 128, 512, 100000, 128, 128, 9
NC = 8
Bc = B // NC          # 16 sequences per core
TOK = Bc * T          # 8192 tokens per core, flat index bt = b*T + t (b-major)
NBLK = TOK // 128     # 64 gather/transpose blocks
G4 = 4
# weight gate order: i, f, g, o (torch).  PSUM column block q per gate:
# i->0, f->1, o->2, g->3  (so sigmoid block i,f,o is contiguous per dir and
# the g block sits at the end; g is handled by the 2*sig(2x)-1 identity).
QMAP = {0: 0, 1: 1, 2: 3, 3: 2}
CH = 32               # LSTM rounds per xproj chunk
NCH = T // CH
DPCH = 32             # viterbi DP precompute chunk (steps)
AM_CH = 56            # bulk-argmax steps per chunk
AM_N = [AM_CH] * 9 + [511 - 9 * AM_CH]
NEG = -1.0e30
NEG16 = -60000.0      # "-inf" that survives fp16 (sigmoid(-6e4+x) == 0)


def build_program():
    nc = bacc.Bacc(None, target_bir_lowering=False)

    # ---------------- dram parameters ----------------
    embed = nc.declare_dram_parameter("embed", [V, E], F32, isOutput=False)
    idx = nc.declare_dram_parameter("idx", [128, NBLK], I32, isOutput=False)
    whh_hi = nc.declare_dram_parameter("whh_hi", [128, 1024], FP16, isOutput=False)
    whh_lo = nc.declare_dram_parameter("whh_lo", [128, 1024], FP16, isOutput=False)
    wih_hi = nc.declare_dram_parameter("wih_hi", [128, 1024], FP16, isOutput=False)
    wih_lo = nc.declare_dram_parameter("wih_lo", [128, 1024], FP16, isOutput=False)
    bias2 = nc.declare_dram_parameter("bias2", [2, 1024], FP16, isOutput=False)
    mneg16 = nc.declare_dram_parameter("mneg16", [1, TOK], FP16, isOutput=False)
    woutT = nc.declare_dram_parameter("woutT", [128, 18], FP16, isOutput=False)
    bout2 = nc.declare_dram_parameter("bout2", [2, K], FP16, isOutput=False)
    start_b = nc.declare_dram_parameter("start_b", [Bc, K], F32, isOutput=False)
    end_b = nc.declare_dram_parameter("end_b", [Bc, K], F32, isOutput=False)
    trans_all = nc.declare_dram_parameter("trans_all", [Bc, 81], F32, isOutput=False)
    irev9_p = nc.declare_dram_parameter("irev9", [Bc, K], F32, isOutput=False)
    iota9_p = nc.declare_dram_parameter("iota9", [Bc, K], F32, isOutput=False)
    mask_dp = nc.declare_dram_parameter("mask_dp", [Bc, T], F32, isOutput=False)
    selneg_p = nc.declare_dram_parameter("selneg", [Bc, T], F32, isOutput=False)
    selAB = nc.declare_dram_parameter("selAB", [48, 144], F32, isOutput=False)
    trans_tiled = nc.declare_dram_parameter("trans_tiled", [K, AM_CH * K], F32, isOutput=False)
    iota_rev_am = nc.declare_dram_parameter("iota_rev_am", [72, AM_CH * K], F32, isOutput=False)
    # col 0: 1056*j (padded-step z value), col 1: 256 + 1024*j (ra2 offset)
    iota_j72 = nc.declare_dram_parameter("iota_j72", [72, 2], F32, isOutput=False)
    mask_bj = nc.declare_dram_parameter("mask_bj", [72, 2 * T], I8, isOutput=False)
    tags_out = nc.declare_dram_parameter("tags", [Bc, T], I32, isOutput=True)

    # ---------------- dram internals ----------------
    xproj_dram = nc.dram_tensor("xproj_dram", [2, G4, Bc, 128, T], FP16)

    with tile.TileContext(nc) as tc:
        with (
            tc.tile_pool(name="big", bufs=1) as big,
            tc.tile_pool(name="xp", bufs=2) as xpp,
            tc.tile_pool(name="consts", bufs=1) as cst,
            tc.tile_pool(name="small", bufs=2) as sm,
            tc.tile_pool(name="pre", bufs=2) as prep,
        ):
            # ---------- constants ----------
            idx_sb = cst.tile([128, NBLK], I32)
            nc.sync.dma_start(out=idx_sb[:], in_=idx[:])
            whhhi_sb = cst.tile([128, 1024], FP16)
            nc.sync.dma_start(out=whhhi_sb[:], in_=whh_hi[:])
            whhlo_sb = cst.tile([128, 1024], FP16)
            nc.sync.dma_start(out=whhlo_sb[:], in_=whh_lo[:])
            wihhi_sb = cst.tile([128, 1024], FP16)
            nc.sync.dma_start(out=wihhi_sb[:], in_=wih_hi[:])
            wihlo_sb = cst.tile([128, 1024], FP16)
            nc.sync.dma_start(out=wihlo_sb[:], in_=wih_lo[:])
            bias_sb = cst.tile([2, 1024], FP16)
            nc.sync.dma_start(out=bias_sb[:], in_=bias2[:])
            mneg_sb = cst.tile([1, TOK], FP16)
            nc.sync.dma_start(out=mneg_sb[:], in_=mneg16[:])
            ones2 = cst.tile([2, 512], FP16)
            nc.vector.memset(ones2[:], 1.0)
            woutT_sb = cst.tile([128, 18], FP16)
            nc.sync.dma_start(out=woutT_sb[:], in_=woutT[:])
            bout_sb = cst.tile([2, K], FP16)
            nc.sync.dma_start(out=bout_sb[:], in_=bout2[:])
            ident = cst.tile([128, 128], F32)
            make_identity(nc, ident[:])
            ident16 = cst.tile([128, 128], FP16)
            nc.vector.tensor_copy(out=ident16[:], in_=ident[:])

            # PE "absorber" ops: self-loading matmuls may carry at most ONE
            # sync wait in walrus codegen.  These tiny ops advance PE's
            # vector clock over one-time deps (identity from Pool,
            # const-weight DMA lanes) so real matmuls each need <=1 wait.
            psp_cm = tc.tile_pool(name="psglob", bufs=1, space="PSUM")
            psp = psp_cm.__enter__()
            pq1 = psp.tile([128, 128], F32, tag="pq1", name="pq1")
            pq2 = psp.tile([128, 128], F32, tag="pq2", name="pq2")
            pw1 = psp.tile([128, 512], F32, tag="pw1", name="pw1")
            pw2 = psp.tile([128, 512], F32, tag="pw2", name="pw2")
            pw3 = psp.tile([128, 512], F32, tag="pw3", name="pw3")
            nc.tensor.transpose(out=pq1[:], in_=ident[:], identity=ident[:])
            for cst_ap in (wihhi_sb[:, 0:1], wihlo_sb[:, 0:1],
                           whhhi_sb[:, 0:1], whhlo_sb[:, 0:1],
                           woutT_sb[:, 0:1], ident16[:, 0:1]):
                nc.tensor.matmul(out=pq2[0:1, 0:1], lhsT=cst_ap,
                                 rhs=cst_ap, start=True, stop=True)
            for cst_ap in (bias_sb[:, 0:1], mneg_sb[:, 0:1], ones2[:, 0:1],
                           bout_sb[:, 0:1]):
                nc.tensor.matmul(out=pq2[0:1, 0:1], lhsT=cst_ap,
                                 rhs=cst_ap, start=True, stop=True)

            # ---------- P0: gather ----------
            x_rows = []
            with tc.tile_pool(name="xr", bufs=12) as xrp:
                for g in range(NBLK):
                    xr = xrp.tile([128, 128], F32, tag="xr")
                    nc.gpsimd.indirect_dma_start(
                        out=xr[:],
                        out_offset=None,
                        in_=embed[:],
                        in_offset=IndirectOffsetOnAxis(
                            ap=idx_sb[:, g:g + 1], axis=0),
                    )
                    x_rows.append(xr)

                # ---------- P1: transpose ----------
                # All PE-facing producers routed through DVE so each
                # self-loading matmul needs a single sync wait (walrus limit):
                # relay gathered blocks DVE, output copies DVE (fp16).
                x_T = big.tile([128, TOK], FP16, tag="xT")
                with tc.tile_pool(name="xrel", bufs=4) as xrelp:
                    psts = [pq1, pq2]
                    for g in range(NBLK):
                        xrel = xrelp.tile([128, 128], F32, tag="xrel")
                        nc.vector.tensor_tensor(
                            out=xrel[:], in0=x_rows[g][:], in1=x_rows[g][:],
                            op=Alu.max)
                        pst = psts[g % 2]
                        nc.tensor.transpose(
                            out=pst[:], in_=xrel[:], identity=ident[:])
                        nc.vector.tensor_copy(
                            out=x_T[:, g * 128:(g + 1) * 128], in_=pst[:])

            # ---------- P2: bulk xproj ----------
            ps2s = [pw1[:], pw2[:], pw3[:]]
            n2 = 0
            for d in range(2):
                for b in range(Bc):
                    for g in range(G4):
                        blk = slice((d * G4 + g) * 128, (d * G4 + g + 1) * 128)
                        ps2 = ps2s[n2 % 3]
                        n2 += 1
                        nc.tensor.matmul(
                            out=ps2, lhsT=wihhi_sb[:, blk],
                            rhs=x_T[:, b * T:(b + 1) * T],
                            start=True, stop=False, skip_group_check=True)
                        nc.tensor.matmul(
                            out=ps2, lhsT=wihlo_sb[:, blk],
                            rhs=x_T[:, b * T:(b + 1) * T],
                            start=False, stop=False, skip_group_check=True)
                        # bias via contract-1 matmul row
                        last = not (d == 1 and g in (0, 1))
                        nc.tensor.matmul(
                            out=ps2, lhsT=bias_sb[:, blk], rhs=ones2[:],
                            start=False, stop=last, skip_group_check=True)
                        if not last:  # bwd i,f: freeze padding via mneg row
                            nc.tensor.matmul(
                                out=ps2, lhsT=ones2[0:1, 0:128],
                                rhs=mneg_sb[:, b * T:(b + 1) * T],
                                start=False, stop=True, skip_group_check=True)
                        xp_sb = sm.tile([128, 512], FP16, tag="xp_out")
                        nc.scalar.copy(out=xp_sb[:], in_=ps2)
                        nc.sync.dma_start(
                            out=xproj_dram[d, QMAP[g], b], in_=xp_sb[:])

            # ---------- P3: LSTM ----------
            h_f = big.tile([128, TOK], FP16, tag="hf")
            h_b = big.tile([128, TOK], FP16, tag="hb")
            h0 = cst.tile([128, Bc], FP16)
            nc.vector.memset(h0[:], 0.0)
            c_st = cst.tile([128, 2 * Bc], F32)
            nc.vector.memset(c_st[:], 0.0)

            ps3d = {0: pq1, 1: pq2}
            xp_tiles = {}
            for r in range(T):
                tf, tb = r, T - 1 - r
                c = r // CH
                if r % CH == 0:
                    for d, cc in ((0, c), (1, NCH - 1 - c)):
                        xt = xpp.tile([128, G4 * Bc * CH], FP16, tag=f"xpc{d}")
                        src = xproj_dram[d][:, :, :, cc * CH:(cc + 1) * CH]
                        src = src.transpose([2, 0, 1, 3])
                        dst = xt[:].rearrange(
                            "p (q b t) -> p q b t", q=G4, b=Bc, t=CH)
                        nc.sync.dma_start(out=dst, in_=src)
                        xp_tiles[d] = xt

                sig = sm.tile([128, 128], F32, tag="sig")
                for d, tt in ((0, tf), (1, tb)):
                    ps3 = ps3d[d]
                    if r == 0:
                        hprev = h0[:]
                    elif d == 0:
                        hprev = h_f[:, tf - 1::T]
                    else:
                        hprev = h_b[:, tb + 1::T]
                    # xproj(+bias, +pad mask) seeds PSUM via identity matmul
                    # (single start=True per bank per round; gates accumulate)
                    xsl = xp_tiles[d][:].rearrange(
                        "p (q b t) -> p q b t", q=G4, b=Bc, t=CH
                    )[:, :, :, tt % CH]
                    nc.tensor.matmul(
                        out=ps3[:, 0:64], lhsT=ident16[:],
                        rhs=xsl, start=True, stop=False, skip_group_check=True)
                    for g in range(G4):
                        blk = slice((d * G4 + g) * 128, (d * G4 + g + 1) * 128)
                        col = QMAP[g] * 16
                        nc.tensor.matmul(
                            out=ps3[:, col:col + Bc],
                            lhsT=whhhi_sb[:, blk], rhs=hprev,
                            start=False, stop=False, skip_group_check=True)
                        nc.tensor.matmul(
                            out=ps3[:, col:col + Bc],
                            lhsT=whhlo_sb[:, blk], rhs=hprev,
                            start=False, stop=(g == G4 - 1),
                            skip_group_check=True)
                    # g block gets a real tanh; i,f,o get sigmoid
                    nc.scalar.activation(
                        out=sig[:, d * 64 + 48:d * 64 + 64], in_=ps3[:, 48:64],
                        func=Act.Tanh)
                    nc.scalar.activation(
                        out=sig[:, d * 64:d * 64 + 48], in_=ps3[:, 0:48],
                        func=Act.Sigmoid)
                sigv = sig[:].rearrange("p (d q b) -> p d q b", d=2, q=4)
                t1 = sm.tile([128, 2 * Bc], F32, tag="t1")
                nc.vector.tensor_tensor(
                    out=t1[:].rearrange("p (d b) -> p d b", d=2),
                    in0=sigv[:, :, 0], in1=sigv[:, :, 3], op=Alu.mult)
                t2 = sm.tile([128, 2 * Bc], F32, tag="t2")
                nc.gpsimd.tensor_tensor(
                    out=t2[:].rearrange("p (d b) -> p d b", d=2),
                    in0=sigv[:, :, 1],
                    in1=c_st[:].rearrange("p (d b) -> p d b", d=2),
                    op=Alu.mult)
                nc.vector.tensor_tensor(
                    out=c_st[:], in0=t1[:], in1=t2[:], op=Alu.add)
                tcx = sm.tile([128, 2 * Bc], F32, tag="tc")
                nc.scalar.activation(out=tcx[:], in_=c_st[:], func=Act.Tanh)
                nc.vector.tensor_tensor(
                    out=h_f[:, tf::T], in0=sigv[:, 0, 2],
                    in1=tcx[:, 0:Bc], op=Alu.mult)
                nc.gpsimd.tensor_tensor(
                    out=h_b[:, tb::T], in0=sigv[:, 1, 2],
                    in1=tcx[:, Bc:2 * Bc], op=Alu.mult)

            # ---------- P4: emissions (staged straight into em_dp) ----------
            em_dp = big.tile([Bc, T * K], F32, tag="em_dp")
            ps4s = [pq2[:, 0:K], pq1[:, 0:K]]
            for ch in range(NBLK):
                ps4 = ps4s[ch % 2]
                nc.tensor.matmul(
                    out=ps4, lhsT=h_f[:, ch * 128:(ch + 1) * 128],
                    rhs=woutT_sb[:, 0:K], start=True, stop=False,
                    skip_group_check=True)
                nc.tensor.matmul(
                    out=ps4, lhsT=h_b[:, ch * 128:(ch + 1) * 128],
                    rhs=woutT_sb[:, K:2 * K], start=False, stop=False,
                    skip_group_check=True)
                nc.tensor.matmul(
                    out=ps4, lhsT=ones2[:, 0:128], rhs=bout_sb[:],
                    start=False, stop=True, skip_group_check=True)
                em_sb = sm.tile([128, K], F32, tag="em_sb")
                nc.scalar.copy(out=em_sb[:], in_=ps4)
                b_ch, t0 = ch // 4, (ch % 4) * 128
                nc.sync.dma_start(
                    out=em_dp[b_ch:b_ch + 1, t0 * K:(t0 + 128) * K],
                    in_=em_sb[:])

            # ---------- P5: Viterbi DP ----------
            trans_sb = cst.tile([Bc, 81], F32)
            nc.sync.dma_start(out=trans_sb[:], in_=trans_all[:])
            irev9_sb = cst.tile([Bc, K], F32)
            nc.sync.dma_start(out=irev9_sb[:], in_=irev9_p[:])
            iota9_sb = cst.tile([Bc, K], F32)
            nc.sync.dma_start(out=iota9_sb[:], in_=iota9_p[:])
            start_sb = cst.tile([Bc, K], F32)
            nc.sync.dma_start(out=start_sb[:], in_=start_b[:])
            end_sb = cst.tile([Bc, K], F32)
            nc.sync.dma_start(out=end_sb[:], in_=end_b[:])
            mask_sb = cst.tile([Bc, T], F32)
            nc.sync.dma_start(out=mask_sb[:], in_=mask_dp[:])
            selneg_sb = cst.tile([Bc, T], F32)
            nc.sync.dma_start(out=selneg_sb[:], in_=selneg_p[:])

            # hist slot t = S_t (unfrozen), t = 0..511
            hist = big.tile([Bc, T * K], F32, tag="hist")
            cand = cst.tile([Bc, 81], F32)
            nc.vector.tensor_tensor(
                out=hist[:, 0:K], in0=em_dp[:, 0:K], in1=start_sb[:],
                op=Alu.add)
            # pre chunks on GpSimd: pre[t][i,j] = trans[j,i] + em[t,i]
            pre_tiles = []
            for pc in range(T // DPCH):
                t0 = pc * DPCH
                n = DPCH if pc > 0 else DPCH - 1  # steps t0..t0+n-1, skip t=0
                s0 = t0 if pc > 0 else 1
                pr = prep.tile([Bc, DPCH * 81], F32, tag="pre")
                nc.gpsimd.tensor_tensor(
                    out=pr[:, 0:n * 81].rearrange(
                        "p (t i j) -> p t i j", i=K, j=K),
                    in0=em_dp[:, s0 * K:(s0 + n) * K].rearrange(
                        "p (t i) -> p t i", i=K).unsqueeze(3).to_broadcast(
                        [Bc, n, K, K]),
                    in1=trans_sb[:].rearrange(
                        "p (i j) -> p i j", i=K).unsqueeze(1).to_broadcast(
                        [Bc, n, K, K]),
                    op=Alu.add)
                pre_tiles.append((pr, s0, n))
                for si in range(n):
                    t = s0 + si
                    nc.vector.tensor_tensor(
                        out=cand[:].rearrange("p (i j) -> p i j", i=K),
                        in0=hist[:, (t - 1) * K:t * K].unsqueeze(1)
                        .to_broadcast([Bc, K, K]),
                        in1=pr[:, si * 81:(si + 1) * 81].rearrange(
                            "p (i j) -> p i j", i=K),
                        op=Alu.add)
                    nc.vector.tensor_reduce(
                        out=hist[:, t * K:(t + 1) * K],
                        in_=cand[:].rearrange("p (i j) -> p i j", i=K),
                        axis=AxX, op=Alu.max)

            # final score: S_fin[b,i] = hist[b, len-1, i] via masked max
            tmps = big.tile([Bc, T * K], F32, tag="em_dp")
            nc.vector.tensor_tensor(
                out=tmps[:].rearrange("p (t i) -> p t i", i=K),
                in0=hist[:].rearrange("p (t i) -> p t i", i=K),
                in1=selneg_sb[:].unsqueeze(2).to_broadcast([Bc, T, K]),
                op=Alu.add)
            S = cst.tile([Bc, K], F32)
            nc.vector.tensor_reduce(
                out=S[:], in_=tmps[:].rearrange("p (t i) -> p i t", i=K),
                axis=AxX, op=Alu.max)

            tags_f = big.tile([Bc, T], F32, tag="tags_f")
            nc.vector.tensor_tensor(
                out=S[:], in0=S[:], in1=end_sb[:], op=Alu.add)
            m1 = sm.tile([Bc, 1], F32, tag="m1")
            nc.vector.tensor_reduce(out=m1[:], in_=S[:], axis=AxX, op=Alu.max)
            eqv = sm.tile([Bc, K], F32, tag="eqv")
            nc.vector.tensor_tensor(
                out=eqv[:], in0=S[:], in1=m1[:].to_broadcast([Bc, K]),
                op=Alu.is_equal)
            nc.vector.tensor_tensor(
                out=eqv[:], in0=eqv[:], in1=irev9_sb[:], op=Alu.mult)
            r1 = sm.tile([Bc, 1], F32, tag="r1")
            nc.vector.tensor_reduce(out=r1[:], in_=eqv[:], axis=AxX, op=Alu.max)
            # tags_f carries 1024*tag during the backtrace
            nc.vector.tensor_scalar(
                out=tags_f[:, T - 1:T], in0=r1[:], scalar1=-1024.0,
                scalar2=8192.0, op0=Alu.mult, op1=Alu.add)

            # ---------- P6: bulk argmax ----------
            selAB_dma = cst.tile([48, 144], F32)
            nc.sync.dma_start(out=selAB_dma[:], in_=selAB[:])
            selAB_sb = cst.tile([48, 144], F32)
            nc.vector.tensor_copy(out=selAB_sb[:], in_=selAB_dma[:])
            ttl_dma = cst.tile([K, AM_CH * K], F32)
            nc.sync.dma_start(out=ttl_dma[:], in_=trans_tiled[:])
            iram_sb = cst.tile([72, AM_CH * K], F32)
            nc.sync.dma_start(out=iram_sb[:], in_=iota_rev_am[:])
            ij72_sb = cst.tile([72, 2], F32)
            nc.sync.dma_start(out=ij72_sb[:], in_=iota_j72[:])
            mask_bj_sb = cst.tile([72, 2 * T], I8)
            nc.sync.dma_start(out=mask_bj_sb[:], in_=mask_bj[:])
            Rrhs = cst.tile([48, AM_CH * K], F32)
            nc.vector.memset(Rrhs[:], 0.0)
            nc.vector.tensor_copy(out=Rrhs[0:K, :], in_=ttl_dma[:])

            # idx_dp: [b, (j, s)] layout, s = 0..510 for steps t = 1..511.
            # Overlays the em_dp buffer (em no longer needed; tmps done).
            idx_full = big.tile([Bc, T * K], F32, tag="em_dp")

            psA = pw1[0:72, 0:AM_CH * K]
            psB = pw2[0:72, 0:AM_CH * K]
            s0 = 0
            for ci, ns in enumerate(AM_N):
                W = ns * K
                nc.vector.tensor_tensor(
                    out=Rrhs[32:48, 0:W],
                    in0=hist[:, s0 * K:(s0 + ns) * K],
                    in1=hist[:, s0 * K:(s0 + ns) * K], op=Alu.max)
                nc.tensor.matmul(out=psA[:, 0:W], lhsT=selAB_sb[:, 0:72],
                                 rhs=Rrhs[:, 0:W], start=True, stop=True)
                nc.tensor.matmul(out=psB[:, 0:W], lhsT=selAB_sb[:, 72:144],
                                 rhs=Rrhs[:, 0:W], start=True, stop=True)

                for hi, psx in ((0, psA), (1, psB)):
                    half = "AB"[hi]
                    sbx = sm.tile([72, AM_CH * K], F32, tag=f"sb{half}")
                    nc.vector.tensor_copy(out=sbx[:, 0:W], in_=psx[:, 0:W])
                    view = sbx[:, 0:W].rearrange("p (t i) -> p t i", i=K)
                    mxa = sm.tile([72, AM_CH], F32, tag=f"mx{half}")
                    nc.vector.tensor_reduce(
                        out=mxa[:, 0:ns], in_=view, axis=AxX, op=Alu.max)
                    eqa = sm.tile([72, AM_CH * K], F32, tag=f"eq{half}")
                    nc.vector.tensor_tensor(
                        out=eqa[:, 0:W].rearrange("p (t i) -> p t i", i=K),
                        in0=view,
                        in1=mxa[:, 0:ns].unsqueeze(2).to_broadcast(
                            [72, ns, K]),
                        op=Alu.is_equal)
                    nc.gpsimd.tensor_tensor(
                        out=eqa[:, 0:W], in0=eqa[:, 0:W],
                        in1=iram_sb[:, 0:W], op=Alu.mult)
                    ra = sm.tile([72, AM_CH], F32, tag=f"r{half}")
                    nc.vector.tensor_reduce(
                        out=ra[:, 0:ns],
                        in_=eqa[:, 0:W].rearrange("p (t i) -> p t i", i=K),
                        axis=AxX, op=Alu.max)
                    # z = 32*idx + 1024*j; idx = 8-ra where valid, j at pads
                    ia = sm.tile([72, AM_CH], F32, tag=f"i{half}")
                    nc.vector.tensor_tensor(
                        out=ia[:, 0:ns],
                        in0=ij72_sb[:, 0:1].to_broadcast([72, ns]),
                        in1=ij72_sb[:, 0:1].to_broadcast([72, ns]), op=Alu.max)
                    ra2 = sm.tile([72, AM_CH], F32, tag=f"r2{half}")
                    nc.vector.tensor_scalar(
                        out=ra2[:, 0:ns], in0=ra[:, 0:ns], scalar1=-32.0,
                        scalar2=ij72_sb[:, 1:2], op0=Alu.mult, op1=Alu.add)
                    nc.vector.copy_predicated(
                        out=ia[:, 0:ns],
                        mask=mask_bj_sb[:, hi * T + s0 + 1:
                                        hi * T + s0 + 1 + ns],
                        data=ra2[:, 0:ns])
                    # regroup [(b,j), t] -> [b, (j, s0+t)] via sbuf dma
                    nc.sync.dma_start(
                        out=idx_full[hi * 8:(hi + 1) * 8, 0:K * 511].rearrange(
                            "p (j s) -> p j s", j=K)[:, :, s0:s0 + ns],
                        in_=ia[:, 0:ns])
                s0 += ns

            # ---------- P7: backtrace ----------
            # z[b,j,s] = 32*idx + 1024*j.  u = 32*(z - 1024*tag_{s+1}) has
            # |u| = 1024*idx at j == tag and |u| >= 24576 elsewhere, so an
            # abs-min reduce yields 1024*tag_s directly: 2 ops per step.
            oh = sm.tile([Bc, K], F32, tag="oh")
            for s in range(T - 2, -1, -1):
                nc.vector.tensor_scalar(
                    out=oh[:],
                    in0=idx_full[:, 0:K * 511].rearrange(
                        "p (j s) -> p j s", j=K)[:, :, s],
                    scalar1=tags_f[:, s + 1:s + 2], scalar2=32.0,
                    op0=Alu.subtract, op1=Alu.mult)
                nc.vector.tensor_reduce(
                    out=tags_f[:, s:s + 1], in_=oh[:], axis=AxX, op=Alu.min,
                    apply_absolute_value=True)
            nc.vector.tensor_tensor(
                out=tags_f[:], in0=tags_f[:], in1=mask_sb[:], op=Alu.mult)
            tags_i = big.tile([Bc, T], I32, tag="tags_i")
            nc.vector.tensor_scalar(
                out=tags_i[:], in0=tags_f[:], scalar1=1.0 / 1024.0,
                scalar2=None, op0=Alu.mult)
            nc.sync.dma_start(out=tags_out[:], in_=tags_i[:])
            psp_cm.__exit__(None, None, None)

    nc.finalize()
    return nc


_NC_CACHE = None


def _get_program():
    global _NC_CACHE
    if _NC_CACHE is None:
        _NC_CACHE = build_program()
    return _NC_CACHE


def _fp16(x):
    return np.asarray(x, np.float16)


def make_in_maps(sentences, lengths, embed, Wih_f, Whh_f, bih_f, bhh_f,
                 Wih_b, Whh_b, bih_b, bhh_b, W_out, b_out, start_t, end_t,
                 trans):
    sentences = np.ascontiguousarray(sentences, dtype=np.int32)
    embed = np.ascontiguousarray(embed, dtype=np.float32)
    lengths = np.asarray(lengths)

    def gscale(w):
        return np.asarray(w, np.float32)

    whh_hi = np.zeros((128, 1024), np.float16)
    whh_lo = np.zeros((128, 1024), np.float16)
    wih_hi = np.zeros((128, 1024), np.float16)
    wih_lo = np.zeros((128, 1024), np.float16)
    bias2_np = np.zeros((2, 1024), np.float16)
    for d, (Wih, Whh, bi, bh) in enumerate(
            ((Wih_f, Whh_f, bih_f, bhh_f), (Wih_b, Whh_b, bih_b, bhh_b))):
        Wihs, Whhs = gscale(Wih), gscale(Whh)
        bsum = gscale((np.asarray(bi) + np.asarray(bh))[:, None])[:, 0]
        for g in range(G4):
            cols = slice((d * G4 + g) * 128, (d * G4 + g + 1) * 128)
            wh = Whhs[g * 128:(g + 1) * 128, :].T
            hi = _fp16(wh)
            whh_hi[:, cols] = hi
            whh_lo[:, cols] = _fp16(wh - np.asarray(hi, np.float32))
            wi = Wihs[g * 128:(g + 1) * 128, :].T
            hi = _fp16(wi)
            wih_hi[:, cols] = hi
            wih_lo[:, cols] = _fp16(wi - np.asarray(hi, np.float32))
            bseg = bsum[g * 128:(g + 1) * 128]
            bhi = _fp16(bseg)
            bias2_np[0, cols] = bhi
            bias2_np[1, cols] = _fp16(bseg - np.asarray(bhi, np.float32))

    W_out = np.asarray(W_out, np.float32)
    woutT = np.zeros((128, 18), np.float16)
    woutT[:, 0:K] = _fp16(W_out[:, :128].T)
    woutT[:, K:2 * K] = _fp16(W_out[:, 128:].T)
    bvec = np.asarray(b_out, np.float32)
    bout_hi = _fp16(bvec)
    bout2_np = np.stack(
        [bout_hi, _fp16(bvec - np.asarray(bout_hi, np.float32))], axis=0)
    start_bc = np.broadcast_to(
        np.asarray(start_t, np.float32)[None, :], (Bc, K)).copy()
    end_bc = np.broadcast_to(
        np.asarray(end_t, np.float32)[None, :], (Bc, K)).copy()

    trans_np = np.asarray(trans, np.float32)
    trans_flat = trans_np.T.reshape(-1)  # [(i,j)] = trans[j,i]
    trans_allv = np.broadcast_to(trans_flat[None, :], (Bc, 81)).copy()
    ii = np.arange(K, dtype=np.float32)
    irev9 = np.broadcast_to((8.0 - ii)[None, :], (Bc, K)).copy()
    iota9_np = np.broadcast_to(ii[None, :], (Bc, K)).copy()

    selAB_np = np.zeros((48, 144), np.float32)
    for half in range(2):
        for m in range(72):
            b_loc, j = divmod(m, K)
            selAB_np[32 + half * 8 + b_loc, half * 72 + m] = 1.0
            selAB_np[j, half * 72 + m] = 1.0
    ttl = np.zeros((K, AM_CH * K), np.float32)
    for jp in range(K):
        ttl[jp] = np.tile(trans_np[:, jp], AM_CH)
    iram = np.broadcast_to(
        np.tile(8.0 - ii, AM_CH)[None, :], (72, AM_CH * K)).copy()
    j72 = np.arange(72, dtype=np.float32) % K
    ij72 = np.stack([1056.0 * j72, 256.0 + 1024.0 * j72], axis=1).copy()
    tt = np.arange(T)

    in_maps = []
    for c in range(NC):
        sl = slice(c * Bc, (c + 1) * Bc)
        sents_c = sentences[sl]
        lens_c = np.asarray(lengths[sl], np.float32)
        idx_np = np.zeros((128, NBLK), np.int32)
        p = np.arange(128)
        for g in range(NBLK):
            bt = g * 128 + p
            idx_np[:, g] = sents_c[bt // T, bt % T]
        mask_np = (tt[None, :] < lens_c[:, None]).astype(np.float32)
        # mneg[(b,t)] = 0 if t < len[b] else -6e4 (fp16)
        mneg16_np = np.where(
            mask_np > 0, 0.0, NEG16).astype(np.float16).reshape(1, TOK)
        selneg_np = np.where(
            tt[None, :] == (lens_c[:, None] - 1), 0.0, NEG).astype(np.float32)
        # mask_bj[(half, b_loc, j), t] = mask[half*8 + b_loc, t]
        mbj = np.repeat(mask_np, K, axis=0).astype(np.int8)  # [144, T]
        mask_bj_np = np.concatenate([mbj[:72], mbj[72:]], axis=1)  # [72, 2T]
        in_maps.append({
            "embed": embed,
            "idx": idx_np,
            "whh_hi": whh_hi, "whh_lo": whh_lo,
            "wih_hi": wih_hi, "wih_lo": wih_lo,
            "bias2": bias2_np,
            "mneg16": mneg16_np,
            "woutT": woutT, "bout2": bout2_np,
            "start_b": start_bc, "end_b": end_bc,
            "trans_all": trans_allv, "irev9": irev9, "iota9": iota9_np,
            "mask_dp": mask_np, "selneg": selneg_np,
            "selAB": selAB_np, "trans_tiled": ttl, "iota_rev_am": iram,
            "iota_j72": ij72, "mask_bj": mask_bj_np,
        })
    return in_maps


def run(inputs, trace=False, **kw):
    nc = _get_program()
    in_maps = make_in_maps(**inputs)
    res = run_bass_kernel_spmd(nc, in_maps, list(range(NC)), trace=trace, **kw)
    tags = np.concatenate([r["tags"] for r in res.results], axis=0)
    return tags.astype(np.int32), res


def kernel(**inputs):
    tags, _ = run(inputs)
    return tags
